# revision 2
# baseline (speedup 1.0000x reference)
"""Trainium2 Bass kernel for nn_MultiHeadAttention_62878321214362.

Problem: B=2, S=2048, D=1024, H=16 heads, DK=64, fp32, mask=all-ones.
  out = softmax((q@Wq.T+bq)(k@Wk.T+bk).T / 8) @ (v@Wv.T+bv) @ Wo.T + bo

Sharding (8 cores): core c -> batch b=c//4, head-group g=c%4 (4 heads each).
Each core computes a partial out-projection y_c = attn_out_g @ Wo[:, g-slice].T;
host sums the 4 partials per batch (the "all-reduce") and adds bo.

Math simplifications (exact up to fp rounding):
  - bk: adds a per-query constant to scores -> softmax-invariant -> dropped.
  - bv: softmax rows sum to 1, so attn@(vh + 1*bv) = attn@vh + 1*bv; the
    1*bv term is folded into the host-side constant: bo + bv @ Wo.T.
  - bq: kept (applied on device as per-partition bias in the transposed
    projection layout).

Device layout (per core), everything "transposed" so no on-chip transposes:
  qhT, khT: [hd=256, S] = W_slice @ x.T   (hd on partitions, 2 tiles of 128)
  vh:       [S, hd]  natural layout, with an appended ones-column per head
            (row 64 of the attnV output then holds the softmax denominators)
  scoresT:  [kpos, q] = khT.T @ qhT  per head
  expT = exp(scoresT/8);  outT[65, q] += vh_ext[kpos].T @ expT[kpos]
  normalize outT rows 0:64 by broadcast(1/row64) (K=1 ones matmul broadcast)
  y[s, :]  = aoT.T @ WoT_slice    (partial; host sums over 4 cores)

Matmuls/storage run in bf16 (ATT_DT) with fp32 PSUM accumulation; bf16 beats
float32r here because f32r's fused 4-byte weight load serializes the PE.

Gen-4 (this version): ACT was the serializing engine (~73% busy: 256 exp
chunks/core).  Now ~40% of exp chunks (spread 13/32) are computed on DVE as
Schraudolph int16 bf16-bit patterns (adds ~3.5e-3 rel absmax, total 5.9e-3
vs the 2e-2 gate); the normalize tail stages both heads' denominators into
one [1,2048] tile (one reciprocal+copy per unit, skinny copies on ACT); y
PSUM evacuations alternate DVE/ACT.  fp8 DoubleRow variants were measured
(2x PE rate) but rejected: +1.6e-2 absmax from e4m3's 12.5% quantization
step overruns the error budget (see kernel2/3 transcript).

Attention inner-loop structure (HW-tuned, see transcript):
  - scores use 4 single-bank [128,512] PSUM tiles/kp (4-slot rotation) with
    the exp (N=512) emitted right after each score MM: gives scores a slot
    lookahead over exp so the S->E->S semaphore chain stays off the critical
    path.
  - attnV is emitted in groups of 2 kp (agroup=2), lagging DELAY=2 kp:
    batching amortizes the PE tile-config switch between K=64 row-split
    score MMs and K=128 attnV MMs (~0.5us/kp on HW when alternating).
  - the normalize tail defers its ones-matmul broadcast + multiply into the
    NEXT unit's kp loop (pending_pb/flush_pb) so PE never stalls on the DVE
    reciprocal chain; pb tiles ride the po pool's slot rotation.
  - phase-3 evacuation is DVE (keeps ACT exp-only); y is stored bf16.
  - with hw_loop, ONE For_i wraps all three phases (loop_phases="all"):
    fewer ~2us back-edge barriers and better cross-phase engine drain.
"""

import numpy as np

B, S, D, H = 2, 2048, 1024, 16
DK = D // H          # 64
HPC = 4              # heads per core
HD = HPC * DK        # 256 per-core head dims
NCORES = 8
KT = D // 128        # 8 k-tiles for projections
ST = S // 128        # 16 s-tiles
SCALE = 1.0 / np.sqrt(np.float32(DK))

ATT_DT = "bf16"   # "bf16" | "f32r"  matmul/storage dtype for x, W, attention
_cache = {}


SCH_B = 16248.6   # Schraudolph bias: 127*128 - mantissa correction


def _build(n_reps=1, hw_loop=0, loop_phases="all", tail_mode="pe",
           att_order="int", delay=2, agroup=2, stag=False, hints=False,
           ph3i=True, sch=True, expn=512, dvefrac=13, tailact=True):
    import concourse.bacc as bacc
    import concourse.mybir as mybir
    import concourse.tile as tile

    F32 = mybir.dt.float32
    F32R = mybir.dt.float32r
    DT = mybir.dt.bfloat16 if ATT_DT == "bf16" else F32R

    nc = bacc.Bacc("TRN2", target_bir_lowering=False, debug=False,
                   num_devices=NCORES)

    xq = nc.dram_tensor("xq", [D, S], DT, kind="ExternalInput").ap()
    xk = nc.dram_tensor("xk", [D, S], DT, kind="ExternalInput").ap()
    xv = nc.dram_tensor("xv", [D, S], DT, kind="ExternalInput").ap()
    wq = nc.dram_tensor("wq", [D, HD], DT, kind="ExternalInput").ap()
    wk = nc.dram_tensor("wk", [D, HD], DT, kind="ExternalInput").ap()
    wv = nc.dram_tensor("wv", [D, HD], DT, kind="ExternalInput").ap()
    wo = nc.dram_tensor("wo", [HD, D], DT, kind="ExternalInput").ap()
    bq = nc.dram_tensor("bq", [128, 2], F32, kind="ExternalInput").ap()
    cst = nc.dram_tensor("cst", [128, 64], DT, kind="ExternalInput").ap()
    zc = nc.dram_tensor("zc", [1, 640], DT, kind="ExternalInput").ap()
    y = nc.dram_tensor("y", [S, D], DT, kind="ExternalOutput").ap()

    with tile.TileContext(nc) as tc:
        with (
            tc.tile_pool(name="pers", bufs=1) as pers,
            tc.tile_pool(name="stream", bufs=5) as stream,
            tc.tile_pool(name="expp", bufs=max(8, 2 * (delay + agroup) + 8)) as expp,
            tc.tile_pool(name="small", bufs=2) as small,
            tc.tile_pool(name="ysb", bufs=5) as ysb,
        ):
            # ---- persistent SBUF tiles ----
            wq_sb = pers.tile([128, KT, HD], DT, tag="wq")
            wk_sb = pers.tile([128, KT, HD], DT, tag="wk")
            wv_sb = pers.tile([128, KT, HD], DT, tag="wv")
            wo_sb = pers.tile([128, 2, D], DT, tag="wo")
            bq_sb = pers.tile([128, 2], F32, tag="bq")
            qhT = pers.tile([128, 2, S], DT, tag="qhT")
            khT = pers.tile([128, 2, S], DT, tag="khT")
            vh = pers.tile([128, ST, HPC, DK + 1], DT, tag="vh")
            aoT = pers.tile([128, 2, S], DT, tag="aoT")
            ones64 = pers.tile([1, 64], DT, tag="ones64")
            zrow = pers.tile([1, 640], DT, tag="zrow")

            nc.sync.dma_start(wq_sb[:], wq.rearrange("(t p) n -> p t n", p=128))
            nc.sync.dma_start(wk_sb[:], wk.rearrange("(t p) n -> p t n", p=128))
            nc.sync.dma_start(wv_sb[:], wv.rearrange("(t p) n -> p t n", p=128))
            nc.sync.dma_start(wo_sb[:], wo.rearrange("(t p) n -> p t n", p=128))
            nc.sync.dma_start(bq_sb[:], bq[:])
            nc.sync.dma_start(ones64[:], cst[0:1, :])
            nc.sync.dma_start(zrow[:], zc[:])
            nc.sync.dma_start(
                vh[:, :, :, DK:DK + 1],
                cst.rearrange("p (a b c) -> p a b c", a=ST, b=HPC))

            import contextlib

            loop_all = loop_phases == "all"

            def loop_ctx(ph=0):
                on = hw_loop and not loop_all and (ph in loop_phases)
                return tc.For_i(0, hw_loop, 1) if on else contextlib.nullcontext()

            def loop_ctx_all():
                if not (hw_loop and loop_all):
                    return contextlib.nullcontext()
                kw = {}
                if stag:
                    kw["staggered_reset"] = True
                if hints:
                    kw["hint_engines"] = (
                        mybir.EngineType.PE,
                        mybir.EngineType.Activation,
                        mybir.EngineType.DVE,
                        mybir.EngineType.SP,
                    )
                return tc.For_i(0, hw_loop, 1, **kw)

            for rep in range(n_reps):
              with loop_ctx_all():
                  # ================= Phase 1: projections =================
                  # 4 half-S PSUM tiles [128,1024] per projection (finer slot
                  # rotation: the next projection's half starts as soon as the
                  # matching half of the previous one is evacuated), evacs
                  # alternate DVE/ACT (ACT is idle during phase 1).
                  with tc.tile_pool(name="pp", bufs=4, space="PSUM") as pp, loop_ctx(1):
                      # vh natural layout: [s, hd] per s-tile, 4 s-tiles per
                      # psum tile.  vh accumulation groups are 256 wide = half
                      # a PSUM bank, so a start=True would clear its
                      # bank-neighbor group: zero the banks once with a
                      # start=True matmul, then accumulate with start=False.
                      pv = [pp.tile([128, 1024], F32, tag="pp", name=f"pv{i}")
                            for i in range(4)]
                      for i in range(4):
                          for bank in range(2):
                              nc.tensor.matmul(
                                  pv[i][:, bank * 512:(bank + 1) * 512],
                                  zrow[0:1, 0:128],
                                  zrow[0:1, 128:640],
                                  start=True, stop=True,
                              )
                      for kt in range(KT):
                          xt = stream.tile([128, S], DT, tag="xv")
                          # xv rides the ACT HW-DGE queue so it streams
                          # concurrently with xq/xk on the SP queue -- the
                          # phase-1 DMA critical path is then just xv.
                          nc.scalar.dma_start(xt[:], xv[kt * 128:(kt + 1) * 128, :])
                          for st in range(ST):
                              nc.tensor.matmul(
                                  pv[st // 4][:, (st % 4) * HD:(st % 4 + 1) * HD],
                                  xt[:, st * 128:(st + 1) * 128],
                                  wv_sb[:, kt, :],
                                  start=False, stop=(kt == KT - 1),
                                  skip_group_check=True,
                              )
                      for i in range(4):
                          src = pv[i][:].rearrange("p (s h d) -> p s h d",
                                                   s=4, h=HPC)
                          dst = vh[:, i * 4:(i + 1) * 4, :, 0:DK]
                          if i % 2 == 0:
                              nc.vector.tensor_copy(dst, src)
                          else:
                              nc.scalar.activation(
                                  dst, src, mybir.ActivationFunctionType.Copy)

                      # qhT[mt] = Wq_sl.T @ q^T   (and +bq at evacuation)
                      pq = [pp.tile([128, 1024], F32, tag="pp", name=f"pq{i}")
                            for i in range(4)]
                      for kt in range(KT):
                          xt = stream.tile([128, S], DT, tag="xq")
                          nc.sync.dma_start(xt[:], xq[kt * 128:(kt + 1) * 128, :])
                          for mt in range(2):
                              for c in range(4):
                                  nc.tensor.matmul(
                                      pq[mt * 2 + c // 2][:, (c % 2) * 512:(c % 2 + 1) * 512],
                                      wq_sb[:, kt, mt * 128:(mt + 1) * 128],
                                      xt[:, c * 512:(c + 1) * 512],
                                      start=(kt == 0), stop=(kt == KT - 1),
                                  )
                      for mt in range(2):
                          for h in range(2):
                              # bias-add must be DVE (ACT Copy rejects AP bias)
                              nc.vector.tensor_scalar_add(
                                  qhT[:, mt, h * 1024:(h + 1) * 1024],
                                  pq[mt * 2 + h][:], bq_sb[:, mt:mt + 1])

                      pk = [pp.tile([128, 1024], F32, tag="pp", name=f"pk{i}")
                            for i in range(4)]
                      for kt in range(KT):
                          xt = stream.tile([128, S], DT, tag="xk")
                          nc.sync.dma_start(xt[:], xk[kt * 128:(kt + 1) * 128, :])
                          for mt in range(2):
                              for c in range(4):
                                  nc.tensor.matmul(
                                      pk[mt * 2 + c // 2][:, (c % 2) * 512:(c % 2 + 1) * 512],
                                      wk_sb[:, kt, mt * 128:(mt + 1) * 128],
                                      xt[:, c * 512:(c + 1) * 512],
                                      start=(kt == 0), stop=(kt == KT - 1),
                                  )
                      for mt in range(2):
                          for h in range(2):
                              nc.scalar.activation(
                                  khT[:, mt, h * 1024:(h + 1) * 1024],
                                  pk[mt * 2 + h][:],
                                  mybir.ActivationFunctionType.Copy)

                  # ================= Phase 2: attention =================
                  with (
                      tc.tile_pool(name="ps", bufs=4 if expn == 512 else 2,
                                   space="PSUM") as ps,
                      tc.tile_pool(name="po", bufs=2, space="PSUM") as po,
                      loop_ctx(2),
                  ):
                      # Head-PAIR processing: the two heads of a pair live at
                      # base_partition 0 and 64 of the same khT tile, so their
                      # K=64 score matmuls target disjoint PE row-groups
                      # (tile_position auto-derives from base_partition) and run
                      # CONCURRENTLY in the array. attnV for unit u is emitted
                      # DELAY kp-units late so its exp is done when PE reaches it.
                      DELAY = delay
                      # pending normalize work: list of (dest, recr) -- the pb
                      # broadcast + multiply is deferred into the NEXT unit's kp
                      # loop so PE never stalls on the DVE reciprocal chain.
                      pending_pb = []

                      def flush_pb():
                          for dest, recr in pending_pb:
                              for c in range(2):
                                  pb = po.tile([64, 512], F32, tag="po",
                                               name="pbb")
                                  nc.tensor.matmul(
                                      pb[:], ones64[:],
                                      recr[:, c * 512:(c + 1) * 512],
                                      start=True, stop=True)
                                  nc.vector.tensor_mul(
                                      dest[:, c * 512:(c + 1) * 512],
                                      dest[:, c * 512:(c + 1) * 512],
                                      pb[:])
                          pending_pb.clear()

                      for mt in range(2):
                          for qh in range(2):
                              q0 = qh * 1024
                              pouts = [None, None]

                              def emit_attnv(u, mt=mt):
                                  ukp, uets = u
                                  if pouts[0] is None:
                                      for e in range(2):
                                          pouts[e] = po.tile(
                                              [65, 1024], F32, tag="po",
                                              name=f"pout{e}")
                                  for e in range(2):
                                      for c in range(2):
                                          nc.tensor.matmul(
                                              pouts[e][:, c * 512:(c + 1) * 512],
                                              vh[:, ukp, 2 * mt + e, :],
                                              uets[e][:, c * 512:(c + 1) * 512],
                                              start=(ukp == 0), stop=(ukp == ST - 1),
                                          )

                              pend = []
                              for kp in range(ST):
                                  # 4 single-bank score tiles per kp: the 4-slot
                                  # rotation gives scores a 2-kp lookahead over
                                  # exp, so the S->E->S semaphore chain never
                                  # becomes the critical path.
                                  pscs = {}
                                  ets = [expp.tile([128, 1024], DT, tag="expT",
                                                   name=f"et{mt}_{qh}_{kp}_{e}")
                                         for e in range(2)]
                                  if att_order == "abatch" and pend and len(pend) > DELAY - 1:
                                      emit_attnv(pend.pop(0))
                                  if expn == 1024:
                                      # 2-slot [128,1024] score tiles, one
                                      # N=1024 exp per head: halves the ACT
                                      # per-instruction init overhead; score
                                      # MM emission keeps the e0/e1 row-pair
                                      # order for PE concurrency.
                                      pt = [ps.tile([128, 1024], F32,
                                                    tag="sc",
                                                    name=f"pscp{mt}_{qh}_{kp}_{e}")
                                            for e in range(2)]
                                      for c in range(2):
                                          for e in range(2):
                                              p0 = e * 64
                                              nc.tensor.matmul(
                                                  pt[e][:, c * 512:(c + 1) * 512],
                                                  khT[p0:p0 + 64, mt, kp * 128:(kp + 1) * 128],
                                                  qhT[p0:p0 + 64, mt, q0 + c * 512:q0 + (c + 1) * 512],
                                                  start=True, stop=True,
                                              )
                                              if c == 1:
                                                  nc.scalar.activation(
                                                      ets[e][:], pt[e][:],
                                                      mybir.ActivationFunctionType.Exp,
                                                      scale=float(SCALE))
                                  for c in range(2 if expn == 512 else 0):
                                      for e in range(2):
                                          p0 = e * 64
                                          psc = ps.tile(
                                              [128, 512], F32, tag="sc",
                                              name=f"psc{mt}_{qh}_{kp}_{e}_{c}")
                                          pscs[(e, c)] = psc
                                          nc.tensor.matmul(
                                              psc[:],
                                              khT[p0:p0 + 64, mt, kp * 128:(kp + 1) * 128],
                                              qhT[p0:p0 + 64, mt, q0 + c * 512:q0 + (c + 1) * 512],
                                              start=True, stop=True,
                                          )
                                          if att_order == "int":
                                              if sch and (kp * 4 + e * 2 + c) * dvefrac % 32 < dvefrac:
                                                  # Schraudolph on DVE: the
                                                  # int16 bits of bf16 exp(x/8)
                                                  # ~= round(x*C + B); frees
                                                  # 1/4 of the ACT exp load.
                                                  dst = ets[e][:, c * 512:
                                                               (c + 1) * 512]
                                                  nc.vector.tensor_scalar(
                                                      dst.bitcast(
                                                          mybir.dt.int16),
                                                      psc[:],
                                                      float(128 * np.log2(np.e)
                                                            * SCALE),
                                                      float(SCH_B),
                                                      op0=mybir.AluOpType.mult,
                                                      op1=mybir.AluOpType.add)
                                              else:
                                                  nc.scalar.activation(
                                                      ets[e][:, c * 512:(c + 1) * 512],
                                                      psc[:],
                                                      mybir.ActivationFunctionType.Exp,
                                                      scale=float(SCALE))
                                  if att_order != "int":
                                      for e in range(2):
                                          for c in range(2):
                                              nc.scalar.activation(
                                                  ets[e][:, c * 512:(c + 1) * 512],
                                                  pscs[(e, c)][:],
                                                  mybir.ActivationFunctionType.Exp,
                                                  scale=float(SCALE))
                                  if kp == 1 and tail_mode == "pe":
                                      flush_pb()
                                  # interleave phase-3 st 0..7 (reads only the
                                  # qh0 half of aoT, final after unit (1,0) +
                                  # its flush) into the last unit's kp loop;
                                  # chunks ride the sc slot rotation.
                                  if ph3i and mt == 1 and qh == 1 and 2 <= kp < 10:
                                      st3 = kp - 2
                                      yt3 = ysb.tile([128, 1024], DT, tag="y",
                                                     name=f"yt{st3}")
                                      for nh in range(2):
                                          pyh = ps.tile([128, 512], F32,
                                                        tag="sc",
                                                        name=f"pyh{st3}_{nh}")
                                          for kt2 in range(2):
                                              nc.tensor.matmul(
                                                  pyh[:],
                                                  aoT[:, kt2, st3 * 128:(st3 + 1) * 128],
                                                  wo_sb[:, kt2, nh * 512:(nh + 1) * 512],
                                                  start=(kt2 == 0), stop=(kt2 == 1),
                                              )
                                          if nh == 0:
                                              nc.vector.tensor_copy(
                                                  yt3[:, nh * 512:(nh + 1) * 512],
                                                  pyh[:])
                                          else:
                                              nc.scalar.activation(
                                                  yt3[:, nh * 512:(nh + 1) * 512],
                                                  pyh[:],
                                                  mybir.ActivationFunctionType.Copy)
                                      nc.sync.dma_start(
                                          y[st3 * 128:(st3 + 1) * 128, :],
                                          yt3[:])
                                  pend.append((kp, ets))
                                  if att_order != "abatch" and len(pend) >= DELAY + agroup:
                                      for _ in range(agroup):
                                          emit_attnv(pend.pop(0))
                              for u in pend:
                                  emit_attnv(u)
                              # Normalize rows 0:64 by 1/row64.  Copy-out first
                              # (frees both pout PSUM slots for the next unit's
                              # attnV ASAP), then reciprocal + broadcast +
                              # multiply.  reciprocal_approx_fast misbehaves on
                              # a PSUM source: stage the sums row through SBUF.
                              dests = []
                              sums2 = small.tile([1, 2048], F32, tag="sums2",
                                                 name="sums2")
                              for e in range(2):
                                  pout = pouts[e]
                                  p0 = e * 64
                                  dest = aoT[p0:p0 + 64, mt, q0:q0 + 1024]
                                  nc.vector.tensor_copy(dest, pout[0:64, :])
                                  sdst = sums2[:, e * 1024:(e + 1) * 1024]
                                  if tailact:
                                      nc.scalar.activation(
                                          sdst, pout[64:65, :],
                                          mybir.ActivationFunctionType.Copy)
                                  else:
                                      nc.vector.tensor_copy(sdst, pout[64:65, :])
                                  dests.append(dest)
                              if tail_mode == "none":
                                  continue
                              recf = small.tile([1, 2048], F32, tag="recf",
                                                name="recf")
                              nc.vector.reciprocal_approx_fast(
                                  out=recf[:], in_=sums2[:])
                              recr = small.tile([1, 2048], DT, tag="recr",
                                                name="recr")
                              if tailact:
                                  nc.scalar.activation(
                                      recr[:], recf[:],
                                      mybir.ActivationFunctionType.Copy)
                              else:
                                  nc.vector.tensor_copy(recr[:], recf[:])
                              for e in range(2):
                                  pending_pb.append(
                                      (dests[e], recr[:, e * 1024:(e + 1) * 1024]))
                      if tail_mode == "pe":
                          flush_pb()

                  # ================= Phase 3: output projection =================
                  # st 0..7 were interleaved into unit (1,1) above
                  with tc.tile_pool(name="py", bufs=2, space="PSUM") as py, loop_ctx(3):
                      for st in range(8 if ph3i else 0, ST):
                          pyt = py.tile([128, 1024], F32, tag="py")
                          for nh in range(2):
                              for kt2 in range(2):
                                  nc.tensor.matmul(
                                      pyt[:, nh * 512:(nh + 1) * 512],
                                      aoT[:, kt2, st * 128:(st + 1) * 128],
                                      wo_sb[:, kt2, nh * 512:(nh + 1) * 512],
                                      start=(kt2 == 0), stop=(kt2 == 1),
                                  )
                          yt = ysb.tile([128, 1024], DT, tag="y")
                          if st % 2 == 0:
                              nc.vector.tensor_copy(yt[:], pyt[:])
                          else:
                              nc.scalar.activation(
                                  yt[:], pyt[:],
                                  mybir.ActivationFunctionType.Copy)
                          nc.sync.dma_start(y[st * 128:(st + 1) * 128, :], yt[:])

    nc.compile()
    return nc


def prepare_in_maps(q, k, v, mask, Wq, bq, Wk, bk, Wv, bv, Wo, bo):
    q = np.asarray(q, dtype=np.float32)
    k = np.asarray(k, dtype=np.float32)
    v = np.asarray(v, dtype=np.float32)
    Wq, Wk, Wv, Wo = (np.asarray(w, dtype=np.float32) for w in (Wq, Wk, Wv, Wo))
    bq, bv, bo = (np.asarray(x, dtype=np.float32) for x in (bq, bv, bo))

    if ATT_DT == "bf16":
        import ml_dtypes
        ndt = ml_dtypes.bfloat16
    else:
        ndt = np.float32
    WqT, WkT, WvT, WoT = Wq.T, Wk.T, Wv.T, Wo.T
    xT = {b: {} for b in range(B)}
    for b in range(B):
        xT[b]["q"] = np.ascontiguousarray(q[b].T.astype(ndt))
        xT[b]["k"] = np.ascontiguousarray(k[b].T.astype(ndt))
        xT[b]["v"] = np.ascontiguousarray(v[b].T.astype(ndt))

    in_maps = []
    for c in range(NCORES):
        b, g = divmod(c, 4)
        hs = g * HD
        in_maps.append({
            "xq": xT[b]["q"],
            "xk": xT[b]["k"],
            "xv": xT[b]["v"],
            "wq": np.ascontiguousarray(WqT[:, hs:hs + HD].astype(ndt)),
            "wk": np.ascontiguousarray(WkT[:, hs:hs + HD].astype(ndt)),
            "wv": np.ascontiguousarray(WvT[:, hs:hs + HD].astype(ndt)),
            "wo": np.ascontiguousarray(WoT[hs:hs + HD, :].astype(ndt)),
            "bq": np.ascontiguousarray(bq[hs:hs + HD].reshape(2, 128).T),
            "cst": np.ones((128, 64), dtype=ndt),
            "zc": np.zeros((1, 640), dtype=ndt),
        })
    return in_maps


def kernel(q, k, v, mask, Wq, bq, Wk, bk, Wv, bv, Wo, bo):
    import os
    # NTFF tracing is unavailable under this axon relay (antenv.axon_hooks
    # missing); make sure an inherited BASS_TRACE can't crash the run.
    os.environ["BASS_NEVER_TRACE"] = "1"
    from concourse.bass_utils import run_bass_kernel_spmd

    if "nc" not in _cache:
        _cache["nc"] = _build()
    nc = _cache["nc"]

    in_maps = prepare_in_maps(q, k, v, mask, Wq, bq, Wk, bk, Wv, bv, Wo, bo)
    Wo = np.asarray(Wo, dtype=np.float32)
    bv = np.asarray(bv, dtype=np.float32)
    bo = np.asarray(bo, dtype=np.float32)

    res = run_bass_kernel_spmd(nc, in_maps, core_ids=list(range(NCORES)))
    _cache["last_results"] = res

    const = (bo + bv @ Wo.T).astype(np.float32)   # folded bv + bo correction
    out = np.empty((B, S, D), dtype=np.float32)
    for b in range(B):
        acc = res.results[4 * b]["y"].astype(np.float32).copy()
        for g in range(1, 4):
            acc += res.results[4 * b + g]["y"]
        out[b] = acc + const
    return out

